# revision 28
# baseline (speedup 1.0000x reference)
"""MiniBatch K-means (1 iteration) on 8 Trainium2 NeuronCores.

Strategy (data-parallel over points, per sharding hint):
  - Shard X along N across 8 cores (62500 points each, zero-padded to
    62720 = 490 tiles of 128 points, 49 slabs of 10 tiles).
  - Per 128-point tile on each core:
      mm1:  q[n,k] = BETA*(c2[k]/2 - x_n.c_k)  (argmin_k == argmin dist),
            two full-rate bf16 matmuls accumulating in PSUM (hi/lo error
            compensation; see below)
      DVE:  m[n] = min_k q[n,k], batched 3 tiles per tensor_reduce over a
            3-bank PSUM group to amortize the ~250-cycle PSUM access
            penalty (575ns/tile vs 658 unbatched)
      onehot[n,k] -> fp8e4 SBUF, two producers interleaved to balance
            engine load:
              11/12 tiles: ACT  exp(m - q)     (=1 at argmin, ~0 elsewhere)
               1/12 tiles: Pool is_equal(q, m) (exact 0/1)
      mm2:  S[65,512] += [X|1]_tile.T @ onehot, as fp8e4 DoubleRow
            matmuls: one DR instruction contracts TWO 128-row slots at
            0.5 cycles/row, so each pair of tiles costs 2 DR matmuls
            (hi and lo x-slots) = 512 cycles instead of 1024.
            x is split x = x8h + x8l (fp8 hi/lo, ~bf16 precision; rel
            err of the final means ~5e-4 from this source).
  - Host: sum the 8 per-core S partials, divide, transpose.

Precision: bf16 matmuls stream at full PE rate but rounding X/C to bf16
perturbs distances by ~4e-3 relative, flipping too many near-boundary
assignments (rel err 8e-2, gate is 2e-2). fp32 matmuls are exact but
stream at 1/4 rate. Instead we split x = xh + xl and c = ch + cl (bf16
hi/lo) and compute
    x.c ~= (xh+xl).ch + xh.cl        (only xl.cl ~ 2^-18 is dropped)
  mm1a: lhsT = [xh^T; xl^T] (128 rows), rhs = [ch; ch]        (start)
  mm1b: lhsT = [xh^T; 1;1;1] (67 rows), rhs = [cl; c2 hi/mid/lo] (stop)
The BETA*c2/2 row is split into three bf16 terms so its absolute error
stays ~2^-24 of its ~2e6 magnitude. Net distance error ~1e-4 in h units,
i.e. fp32-reference-level assignment fidelity at bf16 speed.

Engine budget per tile (steady state, cost-model ns):
  DVE 575 (grouped tensor_reduce min)                          <- bound
  ACT 561 (exp on 11/12 tiles, 612ns each incl PSUM/SBUF access)
  PE  533 (mm1a 512cyc + mm1b 512cyc + DR share 256cyc @2.4GHz)
  Pool 67 (is_equal on 1/12 tiles)

X layouts are packed host-side into per-slab tensors (10 tiles per DMA)
because each dma_start costs ~1.2us of global HWDGE issue time; a tiny
combined "boot" DMA carries tile 0's operands so the first matmul (and
the chain behind it) starts ~2us into the kernel. Padded points have
all-zero [X|1] rows in mm2's operand, so they contribute nothing to S
regardless of their (garbage) onehot row.
"""

import numpy as np

N, D, K = 500000, 64, 512
NCORES = 8
NS = N // NCORES            # 62500 points per core
PT = 128                    # points per tile (partition dim)
TPS = 10                    # tiles per DMA slab
NSLAB = 49                  # 490 tiles exactly
NTP = NSLAB * TPS           # 490 tiles
NPAD = NTP * PT             # 62720 padded points per core
DA = D + 1                  # 65: X augmented with ones column
DAP = 128                   # DA padded to a valid DR weight size (32/64/128)
DH = D + 3                  # 67: xh rows + three ones rows (c2 hi/mid/lo)
XTF = TPS * PT              # 1280 columns of X^T-part per slab
TPG = 2                     # tiles per PSUM reduce group
BETA = 65536.0
NBOOT = 10                  # leading tiles carried by the boot DMAs (slab 0)
NBOOT1 = 2                  # tiles in the first boot DMA (with cha/clb)
SHED = 10**9                # every SHED-th tile's onehot on DVE (disabled)

_CACHE: dict = {}


def _build_nc():
    from contextlib import ExitStack

    import concourse.bacc as bacc
    import concourse.tile as tile
    from concourse import mybir

    f32 = mybir.dt.float32
    bf16 = mybir.dt.bfloat16
    f8e4 = mybir.dt.float8e4

    nc = bacc.Bacc("TRN2", target_bir_lowering=False, debug=False)

    BOOT1A = NBOOT1 * 2 * PT + K         # tiles 0-1 and cha
    BOOT1C = BOOT1A + K                  # ... plus clb
    BOOT2C = (NBOOT - NBOOT1) * 2 * PT   # tiles 2-9
    boot = nc.dram_tensor("boot", [PT, BOOT1C + BOOT2C], bf16, kind="ExternalInput")
    xall = nc.dram_tensor("xall", [PT, NSLAB, XTF], bf16, kind="ExternalInput")
    xht = nc.dram_tensor("xht", [DH, NSLAB, XTF], bf16, kind="ExternalInput")
    xa8 = nc.dram_tensor("xa8", [PT, NSLAB, 2, TPS, DAP], f8e4, kind="ExternalInput")
    sout = nc.dram_tensor("sout", [DA, K], f32, kind="ExternalOutput")

    with tile.TileContext(nc) as tc, ExitStack() as ctx:
        const = ctx.enter_context(tc.tile_pool(name="const", bufs=1))
        ld = ctx.enter_context(tc.tile_pool(name="ld", bufs=1))
        ohp = ctx.enter_context(tc.tile_pool(name="oh", bufs=1))
        mred = ctx.enter_context(tc.tile_pool(name="mred", bufs=1))
        gp = ctx.enter_context(tc.tile_pool(name="g", bufs=1, space="PSUM"))
        sp = ctx.enter_context(tc.tile_pool(name="s", bufs=1, space="PSUM"))

        # PE p-state warmup: the cost model runs the PE at 1/2-1/4 clock
        # until it has been continuously busy for 3us, and a blocked matmul
        # resets the ramp. Dummy rank-1 matmuls with memset operands (ready
        # ~100ns in) keep the PE streaming through the startup DMA window so
        # every real matmul dispatches with its operands already available,
        # at full clock.
        du = const.tile([1, PT], bf16)
        nc.gpsimd.memset(du[:], 0)

        # boot1 carries tiles 0-1 plus the centroid operands in ONE DMA so
        # the first matmul's whole operand set arrives with a single
        # semaphore ~2.7us in; boot2 (tiles 2-9) lands while tiles 0-1 run.
        boot1_sb = const.tile([PT, BOOT1C], bf16)
        nc.sync.dma_start(boot1_sb[:, :BOOT1A], boot[:, :BOOT1A])
        nc.sync.dma_start(boot1_sb[:, BOOT1A:], boot[:, BOOT1A:BOOT1C])
        boot2_sb = const.tile([PT, BOOT2C], bf16)
        nc.sync.dma_start(boot2_sb[:], boot[:, BOOT1C:])
        cha_sb = boot1_sb[:, NBOOT1 * 2 * PT : NBOOT1 * 2 * PT + K]
        clb_sb = boot1_sb[:DH, NBOOT1 * 2 * PT + K :]

        s_ps = sp.tile([DAP, K], f32)  # lives across the whole loop


        NG = NTP - 1           # 489 tiles (the 490th is pure padding)
        # Manual tile rings (instead of per-iteration pool allocs): Tile
        # emits a release-event pair per allocated tile, and those events
        # serialize the DVE/ACT sequencers; reusing fixed tiles keeps the
        # same WAR/RAW tracking without the release machinery.
        #
        # PSUM layout: all 8 banks. s_ps takes one; the other 7 hold q tiles
        # as a [2,2,2,1] rotation of reduce groups (490 = 70 rotations of 7
        # tiles). Pairing the min-reduce over 2-tile groups amortizes its
        # ~250-cycle PSUM access penalty; the leftover single-bank group
        # keeps the write-after-read window at 5 tiles of PE work (~2.7us),
        # which covers the fill->reduce->exp->free chain so the PE never
        # stalls on bank reuse.
        ROT = 7                                  # tiles per rotation
        GSIZES = (1, 1, 1, 1, 1, 1, 1)           # group sizes within a rotation
        GOFF = (0, 1, 2, 3, 4, 5, 6)             # group start offsets
        MB, OB, LB = 8, 4, 4
        g_tiles = [
            gp.tile([PT, gs, K], f32, name=f"g{i}", tag=f"g{i}")
            for i, gs in enumerate(GSIZES)
        ]
        m_ring = [
            mred.tile([PT, 2], f32, name=f"m{i}", tag=f"m{i}") for i in range(MB)
        ]
        oh_ring = [
            ohp.tile([PT, 2, K], f8e4, name=f"oh{i}", tag=f"oh{i}") for i in range(OB)
        ]
        ld_ring = [
            (
                ld.tile([PT, XTF], bf16, name=f"xall{i}", tag=f"xall{i}"),
                ld.tile([DH, XTF], bf16, name=f"xht{i}", tag=f"xht{i}"),
                ld.tile([PT, 2, TPS, DAP], f8e4, name=f"xa8{i}", tag=f"xa8{i}"),
            )
            for i in range(LB)
        ]
        slabs = [None] * NSLAB  # (xall_t, xht_t, xa8_t) per slab
        # Split-stage software pipeline. Emission (= scheduler linearization)
        # per iteration i: mm1(i), reduce(group) once its last tile's mm1 is
        # emitted, onehot(i-DRN) and, after each odd onehot, the pair's two
        # DoubleRow mm2 matmuls. This makes the per-engine program order
        # match the steady-state schedule and keeps Tile's counting-semaphore
        # thresholds tight.
        DRN = 2   # onehot lag behind mm1 (covers the group reduce)
        DRD = 5   # mm2 lag behind mm1: DR matmuls reach the PE only after
                  # their pair's onehots are long done, so the PE never waits
        _J2G = {}  # tile offset within rotation -> (group idx, col in group)
        for gi, (gs, go) in enumerate(zip(GSIZES, GOFF)):
            for c in range(gs):
                _J2G[go + c] = (gi, c)
        tiles = [None] * NG  # i -> (g_ps, col, si, m_t)
        # mm2 emission schedule: pair p fires DRD tiles behind its second
        # onehot in steady state, but the final pairs use the minimal DRN lag
        # so the tail drains ~3 tiles sooner.
        mm2_at = {}
        for p in range((NG + 1) // 2):
            k = min(2 * p + 1, NG - 1)   # second tile, or the lone last tile
            lag = DRD if k < NG - 2 * (DRD - DRN) else DRN
            mm2_at.setdefault(k + lag, []).append(p)
        ngrp = [0]  # global group counter (for m ring)

        def emit_mm1(i):
            si, ti = divmod(i, TPS)
            if ti == 0:
                xall_t, xht_t, xa8_t = ld_ring[si % LB]
                if si > 0:
                    nc.sync.dma_start(xall_t[:], xall[:, si, :])
                    nc.sync.dma_start(xht_t[:], xht[:, si, :])
                slabs[si] = (xall_t, xht_t, xa8_t)
            elif ti == 3:
                if si == NSLAB - 1:
                    pass  # issued early, with slab NSLAB-2
                else:
                    nc.gpsimd.dma_start(slabs[si][2][:], xa8[:, si, :, :, :])
                if si == NSLAB - 2:
                    nxt = ld_ring[(si + 1) % LB]
                    slabs[si + 1] = nxt
                    nc.gpsimd.dma_start(nxt[2][:], xa8[:, si + 1, :, :, :])
            xall_t, xht_t, _ = slabs[si]
            gi, col = _J2G[i % ROT]
            g_ps = g_tiles[gi]
            if i == 0:
                for _ in range(26):
                    nc.tensor.matmul(
                        g_tiles[6][:, 0, :PT], du[:], du[:], start=True, stop=True
                    )
            if i < NBOOT1:
                o = i * 2 * PT
                lhs_a = boot1_sb[:, o : o + PT]
                lhs_b = boot1_sb[:DH, o + PT : o + 2 * PT]
            elif i < NBOOT:
                o = (i - NBOOT1) * 2 * PT
                lhs_a = boot2_sb[:, o : o + PT]
                lhs_b = boot2_sb[:DH, o + PT : o + 2 * PT]
            else:
                lhs_a = xall_t[:, ti * PT : (ti + 1) * PT]
                lhs_b = xht_t[:, ti * PT : (ti + 1) * PT]
            nc.tensor.matmul(g_ps[:, col, :], lhs_a, cha_sb, start=True, stop=False)
            nc.tensor.matmul(g_ps[:, col, :], lhs_b, clb_sb, start=False, stop=True)
            tiles[i] = (g_ps, col, si, None)

        def emit_red(i):
            # reduce the group whose last tile is i
            gi, col = _J2G[i % ROT]
            gs = GSIZES[gi]
            g_ps = g_tiles[gi]
            m_t = m_ring[ngrp[0] % MB]
            ngrp[0] += 1
            nc.vector.tensor_reduce(
                out=m_t[:, :gs],
                in_=g_ps[:],
                axis=mybir.AxisListType.X,
                op=mybir.AluOpType.min,
            )
            for c in range(gs):
                j = i - (gs - 1) + c
                g, cc, si, _ = tiles[j]
                tiles[j] = (g, cc, si, m_t)

        def emit_oh(i):
            g_ps, col, si, m_t = tiles[i]
            p, s = divmod(i, 2)
            dst = oh_ring[p % OB][:, s, :]
            nc.scalar.activation(
                out=dst,
                in_=g_ps[:, col, :],
                func=mybir.ActivationFunctionType.Exp,
                bias=m_t[:, col : col + 1],
                scale=-1.0,
            )

        def emit_mm2(p, first, last):
            si, pi = divmod(p, TPS // 2)
            xa8_t = slabs[si][2]
            oh_t = oh_ring[p % OB]
            nc.tensor.matmul(
                s_ps[:],
                xa8_t[:, 0, 2 * pi : 2 * pi + 2, :],
                oh_t[:],
                start=first,
                stop=False,
                perf_mode=mybir.MatmulPerfMode.DoubleRow,
            )
            nc.tensor.matmul(
                s_ps[:],
                xa8_t[:, 1, 2 * pi : 2 * pi + 2, :],
                oh_t[:],
                start=False,
                stop=last,
                perf_mode=mybir.MatmulPerfMode.DoubleRow,
            )

        for i in range(NG + 3):
            if i < NG:
                emit_mm1(i)
                gi, col = _J2G[i % ROT]
                if col == GSIZES[gi] - 1:
                    emit_red(i)
            k = i - DRN
            if 0 <= k < NG:
                emit_oh(k)
            for p in mm2_at.get(i, ()):
                emit_mm2(p, first=(p == 0), last=(p == (NG + 1) // 2 - 1))

        s_sb = const.tile([DA, K], f32)
        nc.vector.tensor_copy(s_sb[:], s_ps[:DA, :])
        nc.sync.dma_start(sout[:], s_sb[:])

    nc.compile()
    return nc


def _get_nc():
    if "nc" not in _CACHE:
        _CACHE["nc"] = _build_nc()
    return _CACHE["nc"]


def build_in_maps(X, idx):
    import ml_dtypes

    bf = ml_dtypes.bfloat16
    f8 = ml_dtypes.float8_e4m3

    C = X[idx]  # [K, D] float32
    c2 = 0.5 * BETA * np.einsum(
        "kd,kd->k", C.astype(np.float64), C.astype(np.float64)
    )

    cb = (-BETA) * C.T.astype(np.float64)  # [D, K]
    ch = cb.astype(bf)
    cl = (cb - ch.astype(np.float64)).astype(bf)
    c2a = c2.astype(bf)
    c2b = (c2 - c2a.astype(np.float64)).astype(bf)
    c2c = (c2 - c2a.astype(np.float64) - c2b.astype(np.float64)).astype(bf)

    cha_np = np.concatenate([ch, ch], axis=0)  # [128, K]
    clb_np = np.concatenate(
        [cl, c2a[None], c2b[None], c2c[None]], axis=0
    )  # [67, K]

    in_maps = []
    for c in range(NCORES):
        xs = X[c * NS : (c + 1) * NS]  # [NS, D] float32
        xh = xs.astype(bf)
        xl = (xs - xh.astype(np.float32)).astype(bf)

        # [128, NPAD] bf16: rows 0..63 xh^T, rows 64..127 xl^T
        xall_np = np.zeros((PT, NPAD), bf)
        xall_np[:D, :NS] = xh.T
        xall_np[D : 2 * D, :NS] = xl.T
        # [67, NPAD] bf16: rows 0..63 xh^T, rows 64..66 ones
        xht_np = np.zeros((DH, NPAD), bf)
        xht_np[:D, :NS] = xh.T
        xht_np[D:, :NS] = 1.0

        # point-major [X|1] in fp8 hi/lo (pad rows/cols all-zero)
        xa_np = np.zeros((NPAD, DAP), np.float32)
        xa_np[:NS, :D] = xs
        xa_np[:NS, D] = 1.0
        xa_hi = xa_np.astype(f8)
        xa_lo = (xa_np - xa_hi.astype(np.float32)).astype(f8)

        def tile_pm(a):  # [NPAD, DAP] -> [PT, NSLAB, TPS, DAP]
            return np.ascontiguousarray(
                a.reshape(NTP, PT, DAP).transpose(1, 0, 2)
            ).reshape(PT, NSLAB, TPS, DAP)

        xa8_np = np.ascontiguousarray(
            np.stack([tile_pm(xa_hi), tile_pm(xa_lo)], axis=2)
        )  # [PT, NSLAB, 2, TPS, DA]

        NB1C = 2 * 2 * PT + 2 * K
        boot_np = np.zeros((PT, NB1C + 8 * 2 * PT), bf)
        for t in range(NBOOT):
            o = 2 * t * PT if t < 2 else NB1C + 2 * (t - 2) * PT
            boot_np[:, o : o + PT] = xall_np[:, t * PT : (t + 1) * PT]
            boot_np[:DH, o + PT : o + 2 * PT] = xht_np[:, t * PT : (t + 1) * PT]
        boot_np[:, 2 * 2 * PT : 2 * 2 * PT + K] = cha_np
        boot_np[:DH, 2 * 2 * PT + K : NB1C] = clb_np
        in_maps.append(
            {
                "boot": boot_np,
                "xall": xall_np.reshape(PT, NSLAB, XTF),
                "xht": xht_np.reshape(DH, NSLAB, XTF),
                "xa8": xa8_np,
            }
        )
    return in_maps


def kernel(X, init_idx):
    from concourse.bass_utils import run_bass_kernel_spmd

    X = np.ascontiguousarray(np.asarray(X, dtype=np.float32))
    idx = np.asarray(init_idx).astype(np.int64)

    in_maps = build_in_maps(X, idx)
    _CACHE["in_maps"] = in_maps

    # Build a fresh Bass module per call: executing via run_bass_kernel_spmd
    # mutates the module, and re-running a previously-executed one crashes
    # the device (NRT_EXEC_UNIT_UNRECOVERABLE).
    nc = _build_nc()
    res = run_bass_kernel_spmd(nc, in_maps, core_ids=list(range(NCORES)))

    S = np.zeros((DA, K), np.float64)
    for r in res.results:
        S += r["sout"].astype(np.float64)

    counts = S[D, :]                      # [K]
    sums = S[:D, :]                       # [D, K]
    out = (sums / np.maximum(counts, 1.0)).T.astype(np.float32)
    return out
